# revision 12
# baseline (speedup 1.0000x reference)
"""Band-sparse (local block) attention on 8 TRN2 NeuronCores.

Problem: q,k,v [4096, 8, 64] f32; block size 128; banded block mask with 4
blocks each side of the diagonal (window 512). pair_bias is unused.

Sharding: one head per NeuronCore (8 heads / 8 cores). Each core computes
its head's banded attention; host slices/transposes inputs and reassembles
the output.

Per-core algorithm (head h):
  The kernel is ScalarE-bound: every one of the ~4.4M band scores needs an
  exp, and ACT is the only engine with exp (1 elem/cycle/lane @1.2GHz =>
  ~29us of ACTIVATE minimum, plus ~300 cycles of fixed overhead per
  ACTIVATE instruction). To amortize that overhead the per-key-block
  score tiles S^T_c = kT_c.T @ qT[:, band(c)] are packed head-to-tail
  into a continuous "score stream" living in a 6-bank PSUM ring of two
  [128, 1536] windows; ScalarE exps whole windows (23 ACTIVATEs instead
  of 32). QK matmuls split at 512-col PSUM bank boundaries of the
  stream; PV matmuls split wherever a query run crosses a window
  boundary of the P stream.

  PV: for each query group g of 4 row blocks, o_ps[65, 512] accumulates
  vo_c.T @ P_c over the 12 key blocks intersecting the group (vo carries
  a trailing ones column that accumulates the softmax denominator).
  The four catch-up blocks per group are spread one per step to avoid
  PE bursts that would stall the exp stream. Evacuation: DVE copy +
  GpSimd SWDGE DMA (last two groups split across DVE/Sync and
  Scalar/Scalar-HWDGE so the final drain runs in parallel).
  Host: out = (otT[:64] / otT[64:65]).T per head. (Scores ~ N(0,1) after
  the 1/8 scale, so exp without max-subtraction is safe in fp32.)
"""

import os
import sys

import numpy as np


def _ensure_path():
    try:
        import concourse  # noqa: F401
    except ImportError:
        for p in ("/opt/trn_rl_repo", "/root/.axon_site/_ro/trn_rl_repo"):
            if os.path.isdir(p) and p not in sys.path:
                sys.path.insert(0, p)


_ensure_path()

import ml_dtypes  # noqa: E402

import concourse.bacc as bacc  # noqa: E402
import concourse.tile as tile  # noqa: E402
from concourse import mybir  # noqa: E402
from concourse.bass_utils import run_bass_kernel_spmd  # noqa: E402

N, H, D, B = 4096, 8, 64, 128
NROW = N // B  # 32 row/key blocks
BPS = 4  # band: blocks per side
SCALE = 1.0 / 8.0  # D ** -0.5
F32 = mybir.dt.float32
BF16 = mybir.dt.bfloat16
NP_BF16 = ml_dtypes.bfloat16
WIN = 1536  # score-stream window: 3 PSUM banks of fp32
NWIN_BUFS = 2  # PSUM ring: 2 windows = 6 banks (o_ps takes the other 2)
PWIN_BUFS = 8  # SBUF P-stream windows kept live (PV looks back ~6)


def _band(c):
    """Valid query-block range for key block c (inclusive)."""
    return max(0, c - BPS), min(NROW - 1, c + BPS)


def _widths():
    w = []
    for c in range(NROW):
        r_lo, r_hi = _band(c)
        w.append((r_hi - r_lo + 1) * B)
    return w


W_C = _widths()
S_C = [0]
for _w in W_C:
    S_C.append(S_C[-1] + _w)
TOT = S_C[NROW]  # 34304 score-stream columns
NWIN = (TOT + WIN - 1) // WIN  # 23
# Step (key block) whose QK chunks complete each window.
WIN_DONE = [
    min(c for c in range(NROW) if S_C[c + 1] >= min(WIN * (j + 1), TOT))
    for j in range(NWIN)
]


def _build_nc():
    nc = bacc.Bacc(None)
    qt_d = nc.dram_tensor("qt", [D, N], BF16, kind="ExternalInput")
    kt_d = nc.dram_tensor("kt", [D, N], BF16, kind="ExternalInput")
    vo_d = nc.dram_tensor("vo", [B, NROW, D + 1], BF16, kind="ExternalInput")
    ot_d = nc.dram_tensor("ot", [D + 1, N], F32, kind="ExternalOutput")

    with tile.TileContext(nc) as tc:
        with (
            tc.tile_pool(name="io", bufs=1) as io_pool,
            tc.tile_pool(name="pexp", bufs=PWIN_BUFS) as p_pool,
            tc.tile_pool(name="st", bufs=NWIN_BUFS, space="PSUM") as st_pool,
            tc.tile_pool(name="acc", bufs=2, space="PSUM") as acc_pool,
            tc.tile_pool(name="ev", bufs=2) as ev_pool,
        ):
            # HAM warmup: the PE boots throttled to 1.2 GHz and only reaches
            # 2.4 GHz after ~3.4us of sustained activity. Burn dummy matmuls
            # during the initial input-DMA wait so the real stream runs warm.
            wz = io_pool.tile([B, 512], BF16)
            nc.gpsimd.memset(wz, 0.0)
            wps = st_pool.tile([B, WIN], F32, name="st", tag="st")
            for _ in range(10):
                nc.tensor.matmul(
                    wps[:, :512], wz[:, :B], wz, start=True, stop=True
                )

            qt = io_pool.tile([D, N], BF16)
            kt = io_pool.tile([D, N], BF16)
            vo = io_pool.tile([B, NROW, D + 1], BF16)
            # Input DMAs: qt/kt on Sync (HWDGE) with small leading chunks
            # so block 0 is in flight as early as possible, then growing
            # chunks in consumption order; vo rides GpSimd (SWDGE) so its
            # issue cost never queues behind the Sync chunks.
            nc.sync.dma_start(out=kt[:, :256], in_=kt_d[:, :256])
            nc.sync.dma_start(out=qt[:, :768], in_=qt_d[:, :768])
            nc.gpsimd.dma_start(out=vo[:, :16, :], in_=vo_d[:, :16, :])
            nc.sync.dma_start(out=kt[:, 256:1024], in_=kt_d[:, 256:1024])
            nc.sync.dma_start(out=qt[:, 768:1536], in_=qt_d[:, 768:1536])
            nc.sync.dma_start(out=kt[:, 1024:2048], in_=kt_d[:, 1024:2048])
            nc.sync.dma_start(out=qt[:, 1536:2560], in_=qt_d[:, 1536:2560])
            nc.gpsimd.dma_start(out=vo[:, 16:, :], in_=vo_d[:, 16:, :])
            nc.sync.dma_start(out=kt[:, 2048:], in_=kt_d[:, 2048:])
            nc.sync.dma_start(out=qt[:, 2560:], in_=qt_d[:, 2560:])

            st_win = {}  # j -> PSUM window tile
            pc_win = {}  # j -> SBUF exp'd window tile
            o_ps = {}

            def qk(c):
                """Emit block c's QK chunks into the score-stream ring.

                Chunks split at every 512 multiple of the stream so no
                matmul output crosses a PSUM bank boundary; a bank's
                first writer gets start=True (clears the bank), its last
                writer stop=True.
                """
                r_lo, _ = _band(c)
                q_lo = r_lo * B
                a = S_C[c]
                end = S_C[c + 1]
                while a < end:
                    b = min((a // 512 + 1) * 512, end)
                    j = a // WIN
                    if j not in st_win:
                        st_win[j] = st_pool.tile(
                            [B, WIN], F32, name="st", tag="st"
                        )
                    first = a % 512 == 0
                    last = b % 512 == 0 or b == TOT
                    nc.tensor.matmul(
                        st_win[j][:, a - j * WIN : b - j * WIN],
                        kt[:, c * B : (c + 1) * B],
                        qt[:, q_lo + a - S_C[c] : q_lo + b - S_C[c]],
                        start=first,
                        stop=last,
                    )
                    a = b

            def act(j):
                n = min(WIN, TOT - j * WIN)
                pc = p_pool.tile([B, WIN], BF16, tag="pc")
                nc.scalar.activation(
                    pc[:, :n],
                    st_win[j][:, :n],
                    mybir.ActivationFunctionType.Exp,
                    scale=SCALE,
                )
                pc_win[j] = pc

            def pv(g, c, first_call, last_call):
                # accumulate key block c's contribution to query group g.
                # Split rows into runs by "is this row's first
                # contribution" (PSUM overwrite-on-first-touch), then
                # split each run at P-stream window boundaries.
                r_lo = max(4 * g, c - BPS, 0)
                r_hi = min(4 * g + 3, c + BPS, NROW - 1)
                if r_lo > r_hi:
                    return
                q_lo = _band(c)[0] * B
                runs = []
                for r in range(r_lo, r_hi + 1):
                    fresh = c == max(0, r - BPS)
                    if runs and runs[-1][2] == fresh:
                        runs[-1][1] = r
                    else:
                        runs.append([r, r, fresh])
                first_mm = first_call
                n_pieces = []
                for ra, rb, _f in runs:
                    a = S_C[c] + ra * B - q_lo
                    end = S_C[c] + (rb + 1) * B - q_lo
                    while a < end:
                        b = min((a // WIN + 1) * WIN, end)
                        n_pieces.append((a, b))
                        a = b
                for idx, (a, b) in enumerate(n_pieces):
                    j = a // WIN
                    qa = a - S_C[c] + q_lo  # absolute query column
                    nc.tensor.matmul(
                        o_ps[g][:, qa - 4 * g * B : qa - 4 * g * B + (b - a)],
                        vo[:, c, :],
                        pc_win[j][:, a - j * WIN : b - j * WIN],
                        start=first_mm and idx == 0,
                        stop=last_call and idx == len(n_pieces) - 1,
                    )

            def evac(g):
                ev = ev_pool.tile([D + 1, 4 * B], F32, tag="ev")
                out_ap = ot_d[:, 4 * g * B : (4 * g + 4) * B]
                if g == NROW // 4 - 1:
                    # Final group: ScalarE is idle once the last exp is
                    # done; copying + HWDGE-DMAing there runs in parallel
                    # with group 6's DVE copy + Sync DMA instead of
                    # serializing behind them, shortening the drain tail.
                    nc.scalar.copy(ev, o_ps[g])
                    nc.scalar.dma_start(out=out_ap, in_=ev)
                elif g == NROW // 4 - 2:
                    nc.vector.tensor_copy(ev, o_ps[g])
                    nc.sync.dma_start(out=out_ap, in_=ev)
                else:
                    nc.vector.tensor_copy(ev, o_ps[g])
                    nc.gpsimd.dma_start(out=out_ap, in_=ev)

            # Schedule: step c emits block c's QK chunks, then the exp of
            # any window those chunks complete, then PV work whose P data
            # is fully activated. A pv(g, c) waits until block c's last
            # stream window is exp'd (_sched_step), else the PE queue
            # would stall on a ScalarE result that depends on a later PE
            # instruction. All of group g's work starts at step 4g+2 --
            # one step after evac(g-2) frees its PSUM bank -- so the
            # first pv never stalls the PE on the g-2 DVE copy. The four
            # catch-up blocks per group spread over the early steps to
            # keep the PE load even. evac(g) is emitted right after the
            # group's final pv so the copy-out starts as soon as
            # possible.
            plan = _pv_plan()
            for step in range(NROW + 2):
                if step < NROW:
                    qk(step)
                    for j in range(NWIN):
                        if WIN_DONE[j] == step:
                            act(j)
                for g, cc, first_call, last_call in plan.get(step, []):
                    if g not in o_ps:
                        o_ps[g] = acc_pool.tile(
                            [D + 1, 4 * B], F32, name="ops", tag="ops"
                        )
                    pv(g, cc, first_call, last_call)
                    if last_call:
                        evac(g)
                        del o_ps[g]

    nc.compile()
    return nc


def _ready_step(c):
    """First step at which pv work for key block c may be emitted: all
    of block c's stream windows have been exp'd by then (acts are
    emitted at the top of a step, pv afterwards)."""
    return max(c + 1, WIN_DONE[(S_C[c + 1] - 1) // WIN])


def _pv_plan():
    """Static pv schedule: step -> [(g, c, first_call, last_call)].

    Group g's items all land at steps >= 4g+2 (its PSUM bank is freed by
    evac(g-2), emitted at step 4g+1). Catch-up blocks (c < 4g) round-robin
    over the earliest legal steps. Flags mark each group's
    chronologically first/last matmul for PSUM group bookkeeping.
    """
    plan = {}
    for g in range(NROW // 4):
        c_first = max(0, 4 * g - BPS)
        c_last = min(NROW - 1, 4 * g + BPS + 3)
        start = 4 * g + 2
        items = [(max(_ready_step(c), start), c) for c in range(4 * g, c_last + 1)]
        pends = list(range(c_first, 4 * g))
        last_step = max(s for s, _ in items)
        slots = list(range(start, last_step))
        for i, c in enumerate(pends):
            assert slots, f"no pending slots for group {g}"
            s = slots[i % len(slots)]
            assert s >= _ready_step(c)
            items.append((s, c))
        items.sort()
        for i, (s, c) in enumerate(items):
            plan.setdefault(s, []).append(
                (g, c, i == 0, i == len(items) - 1)
            )
    return plan


_NC = None


def _get_nc():
    global _NC
    if _NC is None:
        _NC = _build_nc()
    return _NC


def _make_in_maps(q, k, v):
    q = np.ascontiguousarray(q, dtype=np.float32)
    k = np.ascontiguousarray(k, dtype=np.float32)
    v = np.ascontiguousarray(v, dtype=np.float32)
    in_maps = []
    for h in range(H):
        qT = np.ascontiguousarray(q[:, h, :].T.astype(NP_BF16))  # [64, 4096]
        kT = np.ascontiguousarray(k[:, h, :].T.astype(NP_BF16))
        vb = v[:, h, :].reshape(NROW, B, D).transpose(1, 0, 2)  # [128, 32, 64]
        vo = np.concatenate(
            [vb, np.ones((B, NROW, 1), np.float32)], axis=2
        ).astype(NP_BF16)  # [128, 32, 65]
        in_maps.append(
            {"qt": qT, "kt": kT, "vo": np.ascontiguousarray(vo)}
        )
    return in_maps


def run(q, k, v, trace=False, **trace_kwargs):
    """Returns (out [4096, 8, 64] f32, BassKernelResults)."""
    nc = _get_nc()
    in_maps = _make_in_maps(q, k, v)
    res = run_bass_kernel_spmd(
        nc, in_maps, list(range(H)), trace=trace, **trace_kwargs
    )
    out = np.empty((N, H, D), dtype=np.float32)
    for h in range(H):
        ot = res.results[h]["ot"]  # [65, 4096]
        out[:, h, :] = (ot[:D] / ot[D : D + 1]).T
    return out, res


def kernel(q, k, v, pair_bias=None):
    out, _ = run(q, k, v)
    return out


# revision 14
# speedup vs baseline: 1.0146x; 1.0146x over previous
"""Band-sparse (local block) attention on 8 TRN2 NeuronCores.

Problem: q,k,v [4096, 8, 64] f32; block size 128; banded block mask with 4
blocks each side of the diagonal (window 512). pair_bias is unused.

Sharding: one head per NeuronCore (8 heads / 8 cores). Each core computes
its head's banded attention; host slices/transposes inputs and reassembles
the output.

Per-core algorithm (head h):
  The kernel is ScalarE-bound: every one of the ~4.4M band scores needs an
  exp, and ACT is the only engine with exp (1 elem/cycle/lane @1.2GHz =>
  ~29us of ACTIVATE minimum + ~290ns/instruction overhead). The layout
  keeps the 32-exp stream as gapless as possible and keeps the Scalar
  queue free of everything except the table load and the exps.

  Layout:  qT [64, 4096] (d on partitions), kT [64, 4096],
           vo [128, 32, 65] = per key block j-major V plus a ones column
           (the ones column accumulates the softmax denominator).
  For each key block c (0..31):
    S^T_c = kT_c.T @ qT[:, band(c)]    (PE; [128 keys, W_c<=1152 queries])
    P_c   = exp(S^T_c / 8)             (ACT; PSUM -> SBUF bf16)
  For each query group g of 4 row blocks (0..7), accumulated over the 12
  key blocks intersecting the group's bands:
    o_ps_g [65, 512] += vo_c.T @ P_c[:, group cols]   (PE, PSUM accumulate)
  o_ps rows 0..63 are the unnormalized output^T, row 64 the exp-sums.
  Evacuate via DVE to SBUF, out-DMA via GpSimd SWDGE (Sync keeps the
  input stream, Scalar stays pure).
  Host: out = (outT[:64] / outT[64:65]).T per head. (Scores ~ N(0,1) after
  the 1/8 scale, so exp without max-subtraction is safe in fp32 for this
  input distribution.)
"""

import os
import sys

import numpy as np


def _ensure_path():
    try:
        import concourse  # noqa: F401
    except ImportError:
        for p in ("/opt/trn_rl_repo", "/root/.axon_site/_ro/trn_rl_repo"):
            if os.path.isdir(p) and p not in sys.path:
                sys.path.insert(0, p)


_ensure_path()

import ml_dtypes  # noqa: E402

import concourse.bacc as bacc  # noqa: E402
import concourse.tile as tile  # noqa: E402
from concourse import mybir  # noqa: E402
from concourse.bass_utils import run_bass_kernel_spmd  # noqa: E402

N, H, D, B = 4096, 8, 64, 128
NROW = N // B  # 32 row/key blocks
BPS = 4  # band: blocks per side
SCALE = 1.0 / 8.0  # D ** -0.5
F32 = mybir.dt.float32
BF16 = mybir.dt.bfloat16
NP_BF16 = ml_dtypes.bfloat16
MAXW = (2 * BPS + 1) * B  # 1152: widest band span


def _band(c):
    """Valid query-block range for key block c (inclusive)."""
    return max(0, c - BPS), min(NROW - 1, c + BPS)


def _build_nc():
    nc = bacc.Bacc(None)
    qt_d = nc.dram_tensor("qt", [D, N], BF16, kind="ExternalInput")
    kt_d = nc.dram_tensor("kt", [D, N], BF16, kind="ExternalInput")
    vo_d = nc.dram_tensor("vo", [B, NROW, D + 1], BF16, kind="ExternalInput")
    ot_d = nc.dram_tensor("ot", [D + 1, N], F32, kind="ExternalOutput")

    with tile.TileContext(nc) as tc:
        with (
            tc.tile_pool(name="io", bufs=1) as io_pool,
            tc.tile_pool(name="pexp", bufs=11) as p_pool,
            tc.tile_pool(name="st", bufs=2, space="PSUM") as st_pool,
            tc.tile_pool(name="acc", bufs=2, space="PSUM") as acc_pool,
            tc.tile_pool(name="ev", bufs=2) as ev_pool,
        ):
            # HAM warmup: the PE boots throttled to 1.2 GHz and only reaches
            # 2.4 GHz after ~3.4us of sustained activity. Burn dummy matmuls
            # during the initial input-DMA wait so the real stream runs warm.
            wz = io_pool.tile([B, 512], BF16)
            nc.gpsimd.memset(wz, 0.0)
            wps = st_pool.tile([B, MAXW], F32, name="st", tag="st")
            for _ in range(10):
                nc.tensor.matmul(
                    wps[:, :512], wz[:, :B], wz, start=True, stop=True
                )

            qt = io_pool.tile([D, N], BF16)
            kt = io_pool.tile([D, N], BF16)
            vo = io_pool.tile([B, NROW, D + 1], BF16)
            # Input DMAs: qt/kt on Sync (HWDGE) with small leading chunks
            # so block 0 is in flight as early as possible, then growing
            # chunks in consumption order; vo rides GpSimd (SWDGE) so its
            # issue cost never queues behind the Sync chunks.
            nc.sync.dma_start(out=kt[:, :256], in_=kt_d[:, :256])
            nc.sync.dma_start(out=qt[:, :768], in_=qt_d[:, :768])
            nc.gpsimd.dma_start(out=vo[:, :16, :], in_=vo_d[:, :16, :])
            nc.sync.dma_start(out=kt[:, 256:1024], in_=kt_d[:, 256:1024])
            nc.sync.dma_start(out=qt[:, 768:1536], in_=qt_d[:, 768:1536])
            nc.sync.dma_start(out=kt[:, 1024:2048], in_=kt_d[:, 1024:2048])
            nc.sync.dma_start(out=qt[:, 1536:2560], in_=qt_d[:, 1536:2560])
            nc.gpsimd.dma_start(out=vo[:, 16:, :], in_=vo_d[:, 16:, :])
            nc.sync.dma_start(out=kt[:, 2048:], in_=kt_d[:, 2048:])
            nc.sync.dma_start(out=qt[:, 2560:], in_=qt_d[:, 2560:])

            P = {}  # c -> (sbuf tile of exp scores, q_lo)
            o_ps = {}

            def qk_exp(c):
                r_lo, r_hi = _band(c)
                q_lo = r_lo * B
                w = (r_hi - r_lo + 1) * B
                st = st_pool.tile([B, MAXW], F32, tag="st")
                for off in range(0, w, 512):
                    n = min(512, w - off)
                    nc.tensor.matmul(
                        st[:, off : off + n],
                        kt[:, c * B : (c + 1) * B],
                        qt[:, q_lo + off : q_lo + off + n],
                        start=True,
                        stop=True,
                    )
                pc = p_pool.tile([B, MAXW], BF16, tag="pc")
                nc.scalar.activation(
                    pc[:, :w],
                    st[:, :w],
                    mybir.ActivationFunctionType.Exp,
                    scale=SCALE,
                )
                P[c] = (pc, q_lo)

            def pv(g, c, first_call, last_call):
                # accumulate key block c's contribution to query group g.
                # PSUM group semantics: start=True once per accumulator bank
                # (first matmul; marks the whole 2KB region pending-zero),
                # stop=True on the very last matmul into the bank. Rows
                # joining the accumulation later are handled per element by
                # the PSUM has_written bits (overwrite on first touch,
                # accumulate after), so one matmul can mix fresh and
                # accumulating rows; skip_group_check silences the
                # compile-time uniformity check.
                r_lo = max(4 * g, c - BPS, 0)
                r_hi = min(4 * g + 3, c + BPS, NROW - 1)
                if r_lo > r_hi:
                    return
                pc, q_lo = P[c]
                nc.tensor.matmul(
                    o_ps[g][:, (r_lo - 4 * g) * B : (r_hi + 1 - 4 * g) * B],
                    vo[:, c, :],
                    pc[:, r_lo * B - q_lo : (r_hi + 1) * B - q_lo],
                    start=first_call,
                    stop=last_call,
                    skip_group_check=True,
                )

            def evac(g):
                ev = ev_pool.tile([D + 1, 4 * B], F32, tag="ev")
                out_ap = ot_d[:, 4 * g * B : (4 * g + 4) * B]
                if g == NROW // 4 - 1:
                    # Final group: ScalarE is idle once the last exp is
                    # done; copying + HWDGE-DMAing there runs in parallel
                    # with group 6's DVE copy + Sync DMA instead of
                    # serializing behind them, shortening the drain tail.
                    nc.scalar.copy(ev, o_ps[g])
                    nc.scalar.dma_start(out=out_ap, in_=ev)
                elif g == NROW // 4 - 2:
                    nc.vector.tensor_copy(ev, o_ps[g])
                    nc.sync.dma_start(out=out_ap, in_=ev)
                else:
                    nc.vector.tensor_copy(ev, o_ps[g])
                    nc.gpsimd.dma_start(out=out_ap, in_=ev)

            # Per group g the contributing key blocks are [4g-4, 4g+7].
            # Steady state: block c feeds pv at step c+1 for every group
            # with 4g <= c. The four catch-up blocks (c < 4g, whose P
            # tiles already exist when the group's PSUM bank frees up)
            # are spread one per step over steps 4g+1..4g+4 instead of
            # bursting at 4g+1 -- a burst puts ~2us of PV on the PE in
            # one step, which stalls the next QK and opens a gap in the
            # exp stream.
            for step in range(NROW + 1):
                if step < NROW:
                    qk_exp(step)
                for g in range(NROW // 4):
                    c_first = max(0, 4 * g - BPS)
                    c_last = min(NROW - 1, 4 * g + BPS + 3)
                    first_c = []  # blocks emitted this step, in order
                    if step == 4 * g + 1:
                        o_ps[g] = acc_pool.tile(
                            [D + 1, 4 * B], F32, name="ops", tag="ops"
                        )
                    pend = c_first + (step - (4 * g + 1))
                    if 4 * g + 1 <= step <= 4 * g + 4 and pend < 4 * g:
                        first_c.append(pend)
                    c = step - 1
                    if 4 * g <= c <= c_last and c >= 0:
                        first_c.append(c)
                    for cc in first_c:
                        # c_first is always group g's chronologically first
                        # emitted block (pending slot 0 at step 4g+1, or the
                        # steady block when the band has no catch-up).
                        pv(g, cc, cc == c_first, cc == c_last)
                    if step == c_last + 1:
                        evac(g)

    nc.compile()
    return nc


_NC = None


def _get_nc():
    global _NC
    if _NC is None:
        _NC = _build_nc()
    return _NC


def _make_in_maps(q, k, v):
    q = np.ascontiguousarray(q, dtype=np.float32)
    k = np.ascontiguousarray(k, dtype=np.float32)
    v = np.ascontiguousarray(v, dtype=np.float32)
    in_maps = []
    for h in range(H):
        qT = np.ascontiguousarray(q[:, h, :].T.astype(NP_BF16))  # [64, 4096]
        kT = np.ascontiguousarray(k[:, h, :].T.astype(NP_BF16))
        vb = v[:, h, :].reshape(NROW, B, D).transpose(1, 0, 2)  # [128, 32, 64]
        vo = np.concatenate(
            [vb, np.ones((B, NROW, 1), np.float32)], axis=2
        ).astype(NP_BF16)  # [128, 32, 65]
        in_maps.append(
            {"qt": qT, "kt": kT, "vo": np.ascontiguousarray(vo)}
        )
    return in_maps


def run(q, k, v, trace=False, **trace_kwargs):
    """Returns (out [4096, 8, 64] f32, BassKernelResults)."""
    nc = _get_nc()
    in_maps = _make_in_maps(q, k, v)
    res = run_bass_kernel_spmd(
        nc, in_maps, list(range(H)), trace=trace, **trace_kwargs
    )
    out = np.empty((N, H, D), dtype=np.float32)
    for h in range(H):
        ot = res.results[h]["ot"]  # [65, 4096]
        out[:, h, :] = (ot[:D] / ot[D : D + 1]).T
    return out, res


def kernel(q, k, v, pair_bias=None):
    out, _ = run(q, k, v)
    return out


# revision 15
# speedup vs baseline: 1.4534x; 1.4325x over previous
"""Band-sparse (local block) attention on 8 TRN2 NeuronCores.

Problem: q,k,v [4096, 8, 64] f32; block size 128; banded block mask with 4
blocks each side of the diagonal (window 512). pair_bias is unused.

Sharding: one head per NeuronCore (8 heads / 8 cores). Each core computes
its head's banded attention; host slices/transposes inputs and reassembles
the output.

Per-core algorithm (head h):
  The kernel is ScalarE-bound: every one of the ~4.4M band scores needs an
  exp, and ACT is the only engine with exp (1 elem/cycle/lane @1.2GHz =>
  ~29us of ACTIVATE minimum + ~290ns/instruction overhead). The layout
  keeps the 32-exp stream as gapless as possible and keeps the Scalar
  queue free of everything except the table load and the exps.

  Layout:  qT [64, 4096] (d on partitions), kT [64, 4096],
           vo [128, 32, 65] = per key block j-major V plus a ones column
           (the ones column accumulates the softmax denominator).
  For each key block c (0..31):
    S^T_c = kT_c.T @ qT[:, band(c)]    (PE; [128 keys, W_c<=1152 queries])
    P_c   = exp(S^T_c / 8)             (ACT; PSUM -> SBUF bf16)
  For each query group g of 4 row blocks (0..7), accumulated over the 12
  key blocks intersecting the group's bands:
    o_ps_g [65, 512] += vo_c.T @ P_c[:, group cols]   (PE, PSUM accumulate)
  o_ps rows 0..63 are the unnormalized output^T, row 64 the exp-sums.
  Evacuate via DVE to SBUF, out-DMA via GpSimd SWDGE (Sync keeps the
  input stream, Scalar stays pure).
  Host: out = (outT[:64] / outT[64:65]).T per head. (Scores ~ N(0,1) after
  the 1/8 scale, so exp without max-subtraction is safe in fp32 for this
  input distribution.)
"""

import os
import sys

import numpy as np


def _ensure_path():
    try:
        import concourse  # noqa: F401
    except ImportError:
        for p in ("/opt/trn_rl_repo", "/root/.axon_site/_ro/trn_rl_repo"):
            if os.path.isdir(p) and p not in sys.path:
                sys.path.insert(0, p)


_ensure_path()

import ml_dtypes  # noqa: E402

import concourse.bacc as bacc  # noqa: E402
import concourse.tile as tile  # noqa: E402
from concourse import mybir  # noqa: E402
from concourse.bass_utils import run_bass_kernel_spmd  # noqa: E402

N, H, D, B = 4096, 8, 64, 128
NROW = N // B  # 32 row/key blocks
BPS = 4  # band: blocks per side
SCALE = 1.0 / 8.0  # D ** -0.5
F32 = mybir.dt.float32
BF16 = mybir.dt.bfloat16
NP_BF16 = ml_dtypes.bfloat16
MAXW = (2 * BPS + 1) * B  # 1152: widest band span


def _band(c):
    """Valid query-block range for key block c (inclusive)."""
    return max(0, c - BPS), min(NROW - 1, c + BPS)


def _build_nc():
    nc = bacc.Bacc(None)
    qt_d = nc.dram_tensor("qt", [D, N], BF16, kind="ExternalInput")
    kt_d = nc.dram_tensor("kt", [D, N], BF16, kind="ExternalInput")
    vo_d = nc.dram_tensor("vo", [B, NROW, D + 1], BF16, kind="ExternalInput")
    ot_d = nc.dram_tensor("ot", [D + 1, N], F32, kind="ExternalOutput")

    with tile.TileContext(nc) as tc:
        with (
            tc.tile_pool(name="io", bufs=1) as io_pool,
            tc.tile_pool(name="pexp", bufs=11) as p_pool,
            tc.tile_pool(name="st", bufs=2, space="PSUM") as st_pool,
            tc.tile_pool(name="acc", bufs=2, space="PSUM") as acc_pool,
            tc.tile_pool(name="ev", bufs=2) as ev_pool,
        ):
            # HAM warmup bridge: the PE boots throttled to 1.2 GHz and only
            # reaches 2.4 GHz after ~3.4us of sustained activity -- and it
            # re-throttles (and has been seen to STICK at 1.2 GHz for the
            # whole stream) if it idles again before the real stream
            # starts. So the dummy matmuls must not just warm the PE up,
            # they must bridge the PE from t0 until the first QK's input
            # data has landed (~5us) with no idle gap: ~8 cold matmuls
            # (3.4us) to trip the un-throttle, then warm ones (~213ns
            # each) to cover the remaining DMA wait.
            wz = io_pool.tile([B, 512], BF16)
            nc.gpsimd.memset(wz, 0.0)
            wps = st_pool.tile([B, MAXW], F32, name="st", tag="st")
            for _ in range(16):
                nc.tensor.matmul(
                    wps[:, :512], wz[:, :B], wz, start=True, stop=True
                )

            qt = io_pool.tile([D, N], BF16)
            kt = io_pool.tile([D, N], BF16)
            vo = io_pool.tile([B, NROW, D + 1], BF16)
            # Input DMAs: qt/kt on Sync (HWDGE) with small leading chunks
            # so block 0 is in flight as early as possible, then growing
            # chunks in consumption order; vo rides GpSimd (SWDGE) so its
            # issue cost never queues behind the Sync chunks.
            nc.sync.dma_start(out=kt[:, :256], in_=kt_d[:, :256])
            nc.sync.dma_start(out=qt[:, :768], in_=qt_d[:, :768])
            nc.gpsimd.dma_start(out=vo[:, :16, :], in_=vo_d[:, :16, :])
            nc.sync.dma_start(out=kt[:, 256:1024], in_=kt_d[:, 256:1024])
            nc.sync.dma_start(out=qt[:, 768:1536], in_=qt_d[:, 768:1536])
            nc.sync.dma_start(out=kt[:, 1024:2048], in_=kt_d[:, 1024:2048])
            nc.sync.dma_start(out=qt[:, 1536:2560], in_=qt_d[:, 1536:2560])
            nc.gpsimd.dma_start(out=vo[:, 16:, :], in_=vo_d[:, 16:, :])
            nc.sync.dma_start(out=kt[:, 2048:], in_=kt_d[:, 2048:])
            nc.sync.dma_start(out=qt[:, 2560:], in_=qt_d[:, 2560:])

            P = {}  # c -> (sbuf tile of exp scores, q_lo)
            o_ps = {}

            def qk_exp(c):
                r_lo, r_hi = _band(c)
                q_lo = r_lo * B
                w = (r_hi - r_lo + 1) * B
                st = st_pool.tile([B, MAXW], F32, tag="st")
                for off in range(0, w, 512):
                    n = min(512, w - off)
                    nc.tensor.matmul(
                        st[:, off : off + n],
                        kt[:, c * B : (c + 1) * B],
                        qt[:, q_lo + off : q_lo + off + n],
                        start=True,
                        stop=True,
                    )
                pc = p_pool.tile([B, MAXW], BF16, tag="pc")
                nc.scalar.activation(
                    pc[:, :w],
                    st[:, :w],
                    mybir.ActivationFunctionType.Exp,
                    scale=SCALE,
                )
                P[c] = (pc, q_lo)

            def pv(g, c, first_call, last_call):
                # accumulate key block c's contribution to query group g.
                # PSUM group semantics: start=True once per accumulator bank
                # (first matmul; marks the whole 2KB region pending-zero),
                # stop=True on the very last matmul into the bank. Rows
                # joining the accumulation later are handled per element by
                # the PSUM has_written bits (overwrite on first touch,
                # accumulate after), so one matmul can mix fresh and
                # accumulating rows; skip_group_check silences the
                # compile-time uniformity check.
                r_lo = max(4 * g, c - BPS, 0)
                r_hi = min(4 * g + 3, c + BPS, NROW - 1)
                if r_lo > r_hi:
                    return
                pc, q_lo = P[c]
                nc.tensor.matmul(
                    o_ps[g][:, (r_lo - 4 * g) * B : (r_hi + 1 - 4 * g) * B],
                    vo[:, c, :],
                    pc[:, r_lo * B - q_lo : (r_hi + 1) * B - q_lo],
                    start=first_call,
                    stop=last_call,
                    skip_group_check=True,
                )

            def evac(g):
                ev = ev_pool.tile([D + 1, 4 * B], F32, tag="ev")
                out_ap = ot_d[:, 4 * g * B : (4 * g + 4) * B]
                if g == NROW // 4 - 1:
                    # Final group: ScalarE is idle once the last exp is
                    # done; copying + HWDGE-DMAing there runs in parallel
                    # with group 6's DVE copy + Sync DMA instead of
                    # serializing behind them, shortening the drain tail.
                    nc.scalar.copy(ev, o_ps[g])
                    nc.scalar.dma_start(out=out_ap, in_=ev)
                elif g == NROW // 4 - 2:
                    nc.vector.tensor_copy(ev, o_ps[g])
                    nc.sync.dma_start(out=out_ap, in_=ev)
                else:
                    nc.vector.tensor_copy(ev, o_ps[g])
                    nc.gpsimd.dma_start(out=out_ap, in_=ev)

            # Per group g the contributing key blocks are [4g-4, 4g+7].
            # Steady state: block c feeds pv at step c+1 for every group
            # with 4g <= c. The four catch-up blocks (c < 4g, whose P
            # tiles already exist when the group's PSUM bank frees up)
            # are spread one per step over steps 4g+1..4g+4 instead of
            # bursting at 4g+1 -- a burst puts ~2us of PV on the PE in
            # one step, which stalls the next QK and opens a gap in the
            # exp stream.
            for step in range(NROW + 1):
                if step < NROW:
                    qk_exp(step)
                for g in range(NROW // 4):
                    c_first = max(0, 4 * g - BPS)
                    c_last = min(NROW - 1, 4 * g + BPS + 3)
                    first_c = []  # blocks emitted this step, in order
                    if step == 4 * g + 1:
                        o_ps[g] = acc_pool.tile(
                            [D + 1, 4 * B], F32, name="ops", tag="ops"
                        )
                    pend = c_first + (step - (4 * g + 1))
                    if 4 * g + 1 <= step <= 4 * g + 4 and pend < 4 * g:
                        first_c.append(pend)
                    c = step - 1
                    if 4 * g <= c <= c_last and c >= 0:
                        first_c.append(c)
                    for cc in first_c:
                        # c_first is always group g's chronologically first
                        # emitted block (pending slot 0 at step 4g+1, or the
                        # steady block when the band has no catch-up).
                        pv(g, cc, cc == c_first, cc == c_last)
                    if step == c_last + 1:
                        evac(g)

    nc.compile()
    return nc


_NC = None


def _get_nc():
    global _NC
    if _NC is None:
        _NC = _build_nc()
    return _NC


def _make_in_maps(q, k, v):
    q = np.ascontiguousarray(q, dtype=np.float32)
    k = np.ascontiguousarray(k, dtype=np.float32)
    v = np.ascontiguousarray(v, dtype=np.float32)
    in_maps = []
    for h in range(H):
        qT = np.ascontiguousarray(q[:, h, :].T.astype(NP_BF16))  # [64, 4096]
        kT = np.ascontiguousarray(k[:, h, :].T.astype(NP_BF16))
        vb = v[:, h, :].reshape(NROW, B, D).transpose(1, 0, 2)  # [128, 32, 64]
        vo = np.concatenate(
            [vb, np.ones((B, NROW, 1), np.float32)], axis=2
        ).astype(NP_BF16)  # [128, 32, 65]
        in_maps.append(
            {"qt": qT, "kt": kT, "vo": np.ascontiguousarray(vo)}
        )
    return in_maps


def run(q, k, v, trace=False, **trace_kwargs):
    """Returns (out [4096, 8, 64] f32, BassKernelResults)."""
    nc = _get_nc()
    in_maps = _make_in_maps(q, k, v)
    res = run_bass_kernel_spmd(
        nc, in_maps, list(range(H)), trace=trace, **trace_kwargs
    )
    out = np.empty((N, H, D), dtype=np.float32)
    for h in range(H):
        ot = res.results[h]["ot"]  # [65, 4096]
        out[:, h, :] = (ot[:D] / ot[D : D + 1]).T
    return out, res


def kernel(q, k, v, pair_bias=None):
    out, _ = run(q, k, v)
    return out
